# revision 23
# baseline (speedup 1.0000x reference)
"""PatchAttention Trainium2 kernel (batch-parallel over 8 NeuronCores).

Per core (one batch element):
  v      : patch extraction of x            -> [100 patches, c*ph*pw] in SBUF
  k^T    : 16x16 avgpool(x) @ key_w         -> [256 emb, 100]
  q^T    : conv3x3 s2 (x_high) + BN + SiLU  -> [256 emb, 100]
  score  : per head (d=32)  qT.T kT / sqrt(d) -> softmax -> wts
  y      : wts @ v, computed in per-ph strips [100, (c,pw)] so that both the
           x loads and the out stores are 3-dim DMAs [px, c, pw] per (py, ph).

Self-contained: hardcodes shapes/sharding, imports only concourse + numpy.
"""

import sys

for _p in ("/opt/trn_rl_repo", "/root/.axon_site/_ro/trn_rl_repo"):
    if _p not in sys.path:
        sys.path.append(_p)

import math

import numpy as np

from concourse import bacc, masks, mybir
from concourse.tile import TileContext

F32 = mybir.dt.float32
F32R = mybir.dt.float32r
BF16 = mybir.dt.bfloat16
AX = mybir.AxisListType
AF = mybir.ActivationFunctionType
OP = mybir.AluOpType

# problem constants (per core = one batch element)
C = 128          # low-res channels
HW = 160         # low-res spatial
PH = 16          # patch size
PG = 10          # patch grid (10x10)
P = 100          # patches
HEADS = 8
CH = C // HEADS  # 16 channels per head
EMB = 256
D = EMB // HEADS  # 32
CI = 512         # x_high channels
XH = 20          # x_high spatial
XHP = 22         # padded
EPS = 1e-5
SCL = 1.0 / math.sqrt(D)
NCORES = 8

CONV_DT = BF16   # conv runs in bf16 (BN'd output; error ~1e-3)


def build_nc():
    nc = bacc.Bacc(None, target_bir_lowering=False, debug=False)

    x_t = nc.dram_tensor("x", [C, HW, HW], F32, kind="ExternalInput")
    xh_t = nc.dram_tensor("x_high", [CI, XH, XH], F32, kind="ExternalInput")
    kw_t = nc.dram_tensor("key_w", [C, EMB], F32, kind="ExternalInput")
    cw_t = nc.dram_tensor("conv_w", [EMB, CI, 3, 3], F32, kind="ExternalInput")
    bg_t = nc.dram_tensor("bn_gamma", [EMB], F32, kind="ExternalInput")
    bb_t = nc.dram_tensor("bn_beta", [EMB], F32, kind="ExternalInput")
    bm_t = nc.dram_tensor("bn_mean", [EMB], F32, kind="ExternalInput")
    bv_t = nc.dram_tensor("bn_var", [EMB], F32, kind="ExternalInput")
    out_t = nc.dram_tensor("out", [C, HW, HW], F32, kind="ExternalOutput")

    # dram views: x[c, (py ph), (px pw)] -> [py, px, c, ph, pw]
    x_r = x_t[:].rearrange(
        "c (py ph) (px pw) -> py px c ph pw", py=PG, ph=PH, px=PG, pw=PH
    )
    out_r = out_t[:].rearrange(
        "c (py ph) (px pw) -> py px c ph pw", py=PG, ph=PH, px=PG, pw=PH
    )
    cw_r = cw_t[:].rearrange("o (icc i) kh kw -> o icc (i kh kw)", icc=4)
    xh_r = xh_t[:].rearrange("(icc c) h w -> c icc h w", icc=4)

    with TileContext(nc) as tc:
        with tc.tile_pool(name="const", bufs=1) as const:
            ident_f = const.tile([128, 128], F32, name="ident_f")
            masks.make_identity(nc, ident_f[:])
            if CONV_DT != F32:
                ident_c = const.tile([128, 128], CONV_DT, name="ident_c")
                masks.make_identity(nc, ident_c[:])
            else:
                ident_c = ident_f

            bn_tiles = {}
            for nm, t in (("g", bg_t), ("b", bb_t), ("m", bm_t), ("v", bv_t)):
                bt = const.tile([128, 2], F32, name=f"bn_{nm}")
                nc.sync.dma_start(out=bt[:], in_=t[:].rearrange("(o p) -> p o", p=128))
                bn_tiles[nm] = bt
            keyw = const.tile([128, EMB], F32, name="keyw")
            nc.sync.dma_start(out=keyw[:], in_=kw_t[:])

            # persistent activations
            v_sb = const.tile([128, C * PH * PH], F32, name="v_sb")  # 128 KB
            v4 = v_sb[:].rearrange("p (c ph pw) -> p c ph pw", c=C, ph=PH, pw=PH)
            colsum = const.tile([128, C * PH], F32, name="colsum")  # [p, (c, ph)]
            cs3 = colsum[:].rearrange("p (c ph) -> p c ph", c=C, ph=PH)
            qT_sb = const.tile([128, 2 * P], F32, name="qT_sb")
            kT_sb = const.tile([128, 2 * P], F32, name="kT_sb")
            qT_v = qT_sb[:].rearrange("p (o n) -> p o n", o=2)
            kT_v = kT_sb[:].rearrange("p (o n) -> p o n", o=2)
            pexp = const.tile([128, HEADS * P], F32, name="pexp")
            pexp_v = pexp[:].rearrange("p (h n) -> p h n", h=HEADS)
            se8 = const.tile([128, HEADS], F32, name="se8")  # 1/sum(exp) per head
            zero_t = const.tile([128, 1], F32, name="zero_t")
            nc.vector.memset(zero_t[:], 0.0)

            # ---------------- phase A: conv + BN + SiLU -> qT_sb --------------
            with tc.tile_pool(name="convp", bufs=1) as convp, tc.tile_pool(
                name="wraw", bufs=2
            ) as wraw_pool, tc.tile_pool(
                name="wt_ps", bufs=2, space="PSUM"
            ) as wt_ps_pool, tc.tile_pool(
                name="qt_ps", bufs=2, space="PSUM"
            ) as qt_ps_pool:
                xh_pad = convp.tile([128, 4 * XHP * XHP], CONV_DT, name="xh_pad")
                nc.gpsimd.memset(xh_pad[:], 0.0)
                xh_pad4 = xh_pad[:].rearrange(
                    "p (icc h w) -> p icc h w", icc=4, h=XHP, w=XHP
                )
                for icc in range(4):
                    # SWDGE cast f32 -> CONV_DT during load
                    nc.gpsimd.dma_start(
                        out=xh_pad4[:, icc, 1 : 1 + XH, 1 : 1 + XH],
                        in_=xh_r[:, icc],
                    )
                w_t = convp.tile([128, 4 * 2 * 9 * 128], CONV_DT, name="w_t")
                w_t5 = w_t[:].rearrange(
                    "p (icc occ t o) -> p icc occ t o", icc=4, occ=2, t=9
                )
                for occ in range(2):
                    for icc in range(4):
                        wp = wraw_pool.tile([128, 1152], CONV_DT, name="wp", tag="wp")
                        nc.gpsimd.dma_start(
                            out=wp[:], in_=cw_r[occ * 128 : occ * 128 + 128, icc]
                        )
                        wp3 = wp[:].rearrange("p (i t) -> p i t", t=9)
                        for t in range(9):
                            wt_ps = wt_ps_pool.tile(
                                [128, 128], CONV_DT, name="wt_ps", tag="wt_ps"
                            )
                            nc.tensor.transpose(wt_ps[:], wp3[:, :, t], ident_c[:])
                            nc.vector.tensor_copy(w_t5[:, icc, occ, t, :], wt_ps[:])

                xh6 = xh_pad[:].rearrange(
                    "p (icc rh r2 ch c2) -> p icc rh r2 ch c2",
                    icc=4, rh=11, r2=2, ch=11, c2=2,
                )
                qT_ps = []
                for occ in range(2):
                    qp = qt_ps_pool.tile([128, P], F32, name=f"qT_ps{occ}")
                    qT_ps.append(qp)
                for occ in range(2):
                    n = 0
                    for icc in range(4):
                        for ky in range(3):
                            for kx in range(3):
                                rhs = xh6[
                                    :, icc,
                                    ky // 2 : ky // 2 + 10, ky % 2,
                                    kx // 2 : kx // 2 + 10, kx % 2,
                                ]
                                nc.tensor.matmul(
                                    qT_ps[occ][:],
                                    w_t5[:, icc, occ, ky * 3 + kx, :],
                                    rhs,
                                    start=(n == 0),
                                    stop=(n == 35),
                                )
                                n += 1

                # bn scale/shift: scale = g*rsqrt(v+eps); shift = b - m*scale
                bnw = convp.tile([128, 8], F32, name="bnw")
                lnv, rsq, sc_t, sh_t = (bnw[:, 2 * i : 2 * i + 2] for i in range(4))
                eps_t = convp.tile([128, 1], F32, name="eps_t")
                nc.vector.memset(eps_t[:], EPS)
                nc.scalar.activation(lnv, bn_tiles["v"][:], AF.Ln, bias=eps_t[:])
                nc.scalar.activation(rsq, lnv, AF.Exp, bias=zero_t[:], scale=-0.5)
                nc.vector.tensor_tensor(sc_t, bn_tiles["g"][:], rsq, op=OP.mult)
                nc.vector.tensor_tensor(sh_t, bn_tiles["m"][:], sc_t, op=OP.mult)
                nc.vector.tensor_tensor(sh_t, bn_tiles["b"][:], sh_t, op=OP.subtract)

                # BN + SiLU (silu via exp/recip: one ACT table set for the kernel)
                silu = convp.tile([128, 3 * P], F32, name="silu")
                for occ in range(2):
                    t_sb = silu[:, 0:P]
                    e_sb = silu[:, P : 2 * P]
                    r_sb = silu[:, 2 * P : 3 * P]
                    nc.scalar.activation(
                        t_sb, qT_ps[occ][:], AF.Identity,
                        bias=sh_t[:, occ : occ + 1], scale=sc_t[:, occ : occ + 1],
                    )
                    nc.scalar.activation(e_sb, t_sb, AF.Exp, bias=zero_t[:], scale=-1.0)
                    nc.vector.tensor_scalar_add(e_sb, e_sb, 1.0)
                    nc.vector.reciprocal(r_sb, e_sb)
                    nc.vector.tensor_tensor(qT_v[:, occ, :], t_sb, r_sb, op=OP.mult)

            # ---------------- v loads + pooling + kT --------------------------
            # kT = sum_ph keyw.T @ transpose(pw-sum of v row ph): accumulate
            # per-ph so the whole k path overlaps the v loads.
            nc.vector.tensor_scalar_mul(keyw[:], keyw[:], 1.0 / 256.0)  # avgpool

            with tc.tile_pool(name="pt_ps", bufs=2, space="PSUM") as pt_ps_pool, \
                 tc.tile_pool(name="pt_sb", bufs=2) as pt_sb_pool, \
                 tc.tile_pool(name="kt_ps", bufs=2, space="PSUM") as kt_ps_pool:
                kT_ps = []
                for occ in range(2):
                    kp = kt_ps_pool.tile([128, P], F32, name=f"kT_ps{occ}")
                    kT_ps.append(kp)
                for ph in range(PH):
                    for py in range(PG):
                        r = (ph * PG + py) % 2
                        eng = nc.sync if r == 0 else nc.gpsimd
                        eng.dma_start(
                            out=v4[10 * py : 10 * py + 10, :, ph, :],
                            in_=x_r[py, :, :, ph, :],
                        )
                    # stage-1 pooling: sum over pw for this ph row
                    nc.vector.reduce_sum(
                        cs3[:100, :, ph], v4[:100, :, ph, :], axis=AX.X
                    )
                    pt_ps = pt_ps_pool.tile([128, P], F32, name="pt_ps", tag="ptp")
                    nc.tensor.transpose(
                        pt_ps[:], cs3[:100, :, ph], ident_f[:100, :100]
                    )
                    pt_sb = pt_sb_pool.tile([128, P], F32, name="pt_sb", tag="pts")
                    nc.vector.tensor_copy(pt_sb[:], pt_ps[:])
                    for occ in range(2):
                        nc.tensor.matmul(
                            kT_ps[occ][:],
                            keyw[:, 128 * occ : 128 * occ + 128],
                            pt_sb[:],
                            start=(ph == 0),
                            stop=(ph == PH - 1),
                        )
                for occ in range(2):
                    nc.vector.tensor_copy(kT_v[:, occ, :], kT_ps[occ][:])

                # ---------------- scores + unnormalized softmax ---------------
                # scores are O(1) here so exp needs no max subtraction; the
                # 1/sum normalization is folded into the y strip copies.
                with tc.tile_pool(name="sc_ps", bufs=2, space="PSUM") as sc_ps_pool:
                    for h in range(HEADS):
                        occ, off = h // 4, (h % 4) * 32
                        sc_ps = sc_ps_pool.tile([128, P], F32, name="sc_ps", tag="sc")
                        nc.tensor.matmul(
                            sc_ps[:100],
                            qT_v[off : off + 32, occ, :],
                            kT_v[off : off + 32, occ, :],
                            start=True, stop=True,
                            tile_position=(off, 0),
                        )
                        nc.scalar.activation(
                            pexp_v[:100, h, :], sc_ps[:100],
                            AF.Exp, bias=zero_t[:100], scale=SCL,
                            accum_out=se8[:100, h : h + 1],
                        )
                    nc.vector.reciprocal(se8[:100], se8[:100])

            # ---------------- y = wts @ v in per-ph strips, store -------------
            with tc.tile_pool(name="wt2_sb", bufs=8) as wt2_sb_pool, \
                 tc.tile_pool(name="y_ps", bufs=6, space="PSUM") as y_ps_pool, \
                 tc.tile_pool(name="strip", bufs=4) as strip_pool:
                wtsT = []
                with tc.tile_pool(name="wt2_ps", bufs=2, space="PSUM") as wt2_ps_pool:
                    for h in range(HEADS):
                        wt2_ps = wt2_ps_pool.tile(
                            [128, P], F32, name="wt2_ps", tag="wt2p"
                        )
                        nc.tensor.transpose(
                            wt2_ps[:100], pexp_v[:100, h, :], ident_f[:100, :100]
                        )
                        wt2_sb = wt2_sb_pool.tile(
                            [128, P], F32, name=f"wt2_sb{h}", tag=f"wt2s{h}"
                        )
                        nc.vector.tensor_copy(wt2_sb[:100], wt2_ps[:100])
                        wtsT.append(wt2_sb)
                for ph in range(PH):
                    strip = strip_pool.tile(
                        [128, C * PH], F32, name="strip", tag="strip"
                    )
                    s3 = strip[:].rearrange("p (c pw) -> p c pw", c=C, pw=PH)
                    for h in range(HEADS):
                        y_ps = y_ps_pool.tile([128, CH * PH], F32, name="y_ps",
                                              tag="y_ps")
                        nc.tensor.matmul(
                            y_ps[:100],
                            wtsT[h][:100],
                            v4[:100, CH * h : CH * (h + 1), ph, :],
                            start=True, stop=True,
                        )
                        # copy + softmax normalization (per-q scale)
                        nc.scalar.mul(
                            s3[:100, CH * h : CH * (h + 1), :], y_ps[:100],
                            se8[:100, h : h + 1],
                        )
                    for py in range(PG):
                        eng = nc.sync if py % 2 == 0 else nc.scalar
                        eng.dma_start(
                            out=out_r[py, :, :, ph, :],
                            in_=s3[10 * py : 10 * py + 10],
                        )

    nc.compile()
    return nc


_NC = None


def _get_nc():
    global _NC
    if _NC is None:
        _NC = build_nc()
    return _NC


def _run(inputs: dict, trace: bool = False):
    from concourse.bass_utils import run_bass_kernel_spmd

    nc = _get_nc()
    per_core = []
    for i in range(NCORES):
        m = {
            "x": np.ascontiguousarray(inputs["x"][i], dtype=np.float32),
            "x_high": np.ascontiguousarray(inputs["x_high"][i], dtype=np.float32),
            "key_w": np.ascontiguousarray(inputs["key_w"], dtype=np.float32),
            "conv_w": np.ascontiguousarray(inputs["conv_w"], dtype=np.float32),
            "bn_gamma": np.ascontiguousarray(inputs["bn_gamma"], dtype=np.float32),
            "bn_beta": np.ascontiguousarray(inputs["bn_beta"], dtype=np.float32),
            "bn_mean": np.ascontiguousarray(inputs["bn_mean"], dtype=np.float32),
            "bn_var": np.ascontiguousarray(inputs["bn_var"], dtype=np.float32),
        }
        per_core.append(m)
    res = run_bass_kernel_spmd(nc, per_core, list(range(NCORES)), trace=trace)
    out = np.stack([res.results[i]["out"] for i in range(NCORES)], axis=0)
    return out, res


def kernel(**inputs) -> np.ndarray:
    out, _ = _run(inputs, trace=False)
    return out


# revision 24
# speedup vs baseline: 1.1111x; 1.1111x over previous
"""PatchAttention Trainium2 kernel (batch-parallel over 8 NeuronCores).

Per core (one batch element):
  v      : patch extraction of x            -> [100 patches, c*ph*pw] in SBUF
  k^T    : 16x16 avgpool(x) @ key_w         -> [256 emb, 100]
  q^T    : conv3x3 s2 (x_high) + BN + SiLU  -> [256 emb, 100]
  score  : per head (d=32)  qT.T kT / sqrt(d) -> softmax -> wts
  y      : wts @ v, computed in per-ph strips [100, (c,pw)] so that both the
           x loads and the out stores are 3-dim DMAs [px, c, pw] per (py, ph).

Self-contained: hardcodes shapes/sharding, imports only concourse + numpy.
"""

import sys

for _p in ("/opt/trn_rl_repo", "/root/.axon_site/_ro/trn_rl_repo"):
    if _p not in sys.path:
        sys.path.append(_p)

import math

import numpy as np

from concourse import bacc, masks, mybir
from concourse.tile import TileContext

F32 = mybir.dt.float32
F32R = mybir.dt.float32r
BF16 = mybir.dt.bfloat16
AX = mybir.AxisListType
AF = mybir.ActivationFunctionType
OP = mybir.AluOpType

# problem constants (per core = one batch element)
C = 128          # low-res channels
HW = 160         # low-res spatial
PH = 16          # patch size
PG = 10          # patch grid (10x10)
P = 100          # patches
HEADS = 8
CH = C // HEADS  # 16 channels per head
EMB = 256
D = EMB // HEADS  # 32
CI = 512         # x_high channels
XH = 20          # x_high spatial
XHP = 22         # padded
EPS = 1e-5
SCL = 1.0 / math.sqrt(D)
NCORES = 8

CONV_DT = BF16   # conv runs in bf16 (BN'd output; error ~1e-3)


def build_nc():
    nc = bacc.Bacc(None, target_bir_lowering=False, debug=False)

    x_t = nc.dram_tensor("x", [C, HW, HW], F32, kind="ExternalInput")
    xh_t = nc.dram_tensor("x_high", [CI, XH, XH], F32, kind="ExternalInput")
    kw_t = nc.dram_tensor("key_w", [C, EMB], F32, kind="ExternalInput")
    cw_t = nc.dram_tensor("conv_w", [EMB, CI, 3, 3], F32, kind="ExternalInput")
    bg_t = nc.dram_tensor("bn_gamma", [EMB], F32, kind="ExternalInput")
    bb_t = nc.dram_tensor("bn_beta", [EMB], F32, kind="ExternalInput")
    bm_t = nc.dram_tensor("bn_mean", [EMB], F32, kind="ExternalInput")
    bv_t = nc.dram_tensor("bn_var", [EMB], F32, kind="ExternalInput")
    out_t = nc.dram_tensor("out", [C, HW, HW], F32, kind="ExternalOutput")

    # dram views: x[c, (py ph), (px pw)] -> [py, px, c, ph, pw]
    x_r = x_t[:].rearrange(
        "c (py ph) (px pw) -> py px c ph pw", py=PG, ph=PH, px=PG, pw=PH
    )
    out_r = out_t[:].rearrange(
        "c (py ph) (px pw) -> py px c ph pw", py=PG, ph=PH, px=PG, pw=PH
    )
    cw_r = cw_t[:].rearrange("o (icc i) kh kw -> o icc (i kh kw)", icc=4)
    xh_r = xh_t[:].rearrange("(icc c) h w -> c icc h w", icc=4)

    with TileContext(nc) as tc:
        with tc.tile_pool(name="const", bufs=1) as const:
            ident_f = const.tile([128, 128], F32, name="ident_f")
            masks.make_identity(nc, ident_f[:])
            if CONV_DT != F32:
                ident_c = const.tile([128, 128], CONV_DT, name="ident_c")
                masks.make_identity(nc, ident_c[:])
            else:
                ident_c = ident_f

            bn_tiles = {}
            for nm, t in (("g", bg_t), ("b", bb_t), ("m", bm_t), ("v", bv_t)):
                bt = const.tile([128, 2], F32, name=f"bn_{nm}")
                nc.sync.dma_start(out=bt[:], in_=t[:].rearrange("(o p) -> p o", p=128))
                bn_tiles[nm] = bt
            keyw = const.tile([128, EMB], F32, name="keyw")
            nc.sync.dma_start(out=keyw[:], in_=kw_t[:])

            # persistent activations
            v_sb = const.tile([128, C * PH * PH], F32, name="v_sb")  # 128 KB
            v4 = v_sb[:].rearrange("p (c ph pw) -> p c ph pw", c=C, ph=PH, pw=PH)
            colsum = const.tile([128, C * PH], F32, name="colsum")  # [p, (c, ph)]
            cs3 = colsum[:].rearrange("p (c ph) -> p c ph", c=C, ph=PH)
            qT_sb = const.tile([128, 2 * P], F32, name="qT_sb")
            kT_sb = const.tile([128, 2 * P], F32, name="kT_sb")
            qT_v = qT_sb[:].rearrange("p (o n) -> p o n", o=2)
            kT_v = kT_sb[:].rearrange("p (o n) -> p o n", o=2)
            pexp = const.tile([128, HEADS * P], F32, name="pexp")
            pexp_v = pexp[:].rearrange("p (h n) -> p h n", h=HEADS)
            se8 = const.tile([128, HEADS], F32, name="se8")  # 1/sum(exp) per head
            zero_t = const.tile([128, 1], F32, name="zero_t")
            nc.vector.memset(zero_t[:], 0.0)

            # ---------------- phase A: conv + BN + SiLU -> qT_sb --------------
            with tc.tile_pool(name="convp", bufs=1) as convp, tc.tile_pool(
                name="wraw", bufs=2
            ) as wraw_pool, tc.tile_pool(
                name="wt_ps", bufs=2, space="PSUM"
            ) as wt_ps_pool, tc.tile_pool(
                name="qt_ps", bufs=2, space="PSUM"
            ) as qt_ps_pool:
                xh_pad = convp.tile([128, 4 * XHP * XHP], CONV_DT, name="xh_pad")
                nc.gpsimd.memset(xh_pad[:], 0.0)
                xh_pad4 = xh_pad[:].rearrange(
                    "p (icc h w) -> p icc h w", icc=4, h=XHP, w=XHP
                )
                for icc in range(4):
                    # SWDGE cast f32 -> CONV_DT during load
                    nc.gpsimd.dma_start(
                        out=xh_pad4[:, icc, 1 : 1 + XH, 1 : 1 + XH],
                        in_=xh_r[:, icc],
                    )
                w_t = convp.tile([128, 4 * 2 * 9 * 128], CONV_DT, name="w_t")
                w_t5 = w_t[:].rearrange(
                    "p (icc occ t o) -> p icc occ t o", icc=4, occ=2, t=9
                )
                for occ in range(2):
                    for icc in range(4):
                        wp = wraw_pool.tile([128, 1152], CONV_DT, name="wp", tag="wp")
                        nc.gpsimd.dma_start(
                            out=wp[:], in_=cw_r[occ * 128 : occ * 128 + 128, icc]
                        )
                        wp3 = wp[:].rearrange("p (i t) -> p i t", t=9)
                        for t in range(9):
                            wt_ps = wt_ps_pool.tile(
                                [128, 128], CONV_DT, name="wt_ps", tag="wt_ps"
                            )
                            nc.tensor.transpose(wt_ps[:], wp3[:, :, t], ident_c[:])
                            nc.vector.tensor_copy(w_t5[:, icc, occ, t, :], wt_ps[:])

                xh6 = xh_pad[:].rearrange(
                    "p (icc rh r2 ch c2) -> p icc rh r2 ch c2",
                    icc=4, rh=11, r2=2, ch=11, c2=2,
                )
                qT_ps = []
                for occ in range(2):
                    qp = qt_ps_pool.tile([128, P], F32, name=f"qT_ps{occ}")
                    qT_ps.append(qp)
                for occ in range(2):
                    n = 0
                    for icc in range(4):
                        for ky in range(3):
                            for kx in range(3):
                                rhs = xh6[
                                    :, icc,
                                    ky // 2 : ky // 2 + 10, ky % 2,
                                    kx // 2 : kx // 2 + 10, kx % 2,
                                ]
                                nc.tensor.matmul(
                                    qT_ps[occ][:],
                                    w_t5[:, icc, occ, ky * 3 + kx, :],
                                    rhs,
                                    start=(n == 0),
                                    stop=(n == 35),
                                )
                                n += 1

                # bn scale/shift: scale = g*rsqrt(v+eps); shift = b - m*scale
                bnw = convp.tile([128, 8], F32, name="bnw")
                lnv, rsq, sc_t, sh_t = (bnw[:, 2 * i : 2 * i + 2] for i in range(4))
                eps_t = convp.tile([128, 1], F32, name="eps_t")
                nc.vector.memset(eps_t[:], EPS)
                nc.scalar.activation(lnv, bn_tiles["v"][:], AF.Ln, bias=eps_t[:])
                nc.scalar.activation(rsq, lnv, AF.Exp, bias=zero_t[:], scale=-0.5)
                nc.vector.tensor_tensor(sc_t, bn_tiles["g"][:], rsq, op=OP.mult)
                nc.vector.tensor_tensor(sh_t, bn_tiles["m"][:], sc_t, op=OP.mult)
                nc.vector.tensor_tensor(sh_t, bn_tiles["b"][:], sh_t, op=OP.subtract)

                # BN + SiLU (silu via exp/recip: one ACT table set for the kernel)
                silu = convp.tile([128, 3 * P], F32, name="silu")
                for occ in range(2):
                    t_sb = silu[:, 0:P]
                    e_sb = silu[:, P : 2 * P]
                    r_sb = silu[:, 2 * P : 3 * P]
                    nc.scalar.activation(
                        t_sb, qT_ps[occ][:], AF.Identity,
                        bias=sh_t[:, occ : occ + 1], scale=sc_t[:, occ : occ + 1],
                    )
                    nc.scalar.activation(e_sb, t_sb, AF.Exp, bias=zero_t[:], scale=-1.0)
                    nc.vector.tensor_scalar_add(e_sb, e_sb, 1.0)
                    nc.vector.reciprocal(r_sb, e_sb)
                    nc.vector.tensor_tensor(qT_v[:, occ, :], t_sb, r_sb, op=OP.mult)

            # ---------------- v loads + pooling + kT --------------------------
            # kT = sum_ph keyw.T @ transpose(pw-sum of v row ph): accumulate
            # per-ph so the whole k path overlaps the v loads.
            nc.vector.tensor_scalar_mul(keyw[:], keyw[:], 1.0 / 256.0)  # avgpool

            with tc.tile_pool(name="pt_ps", bufs=2, space="PSUM") as pt_ps_pool, \
                 tc.tile_pool(name="pt_sb", bufs=2) as pt_sb_pool, \
                 tc.tile_pool(name="kt_ps", bufs=2, space="PSUM") as kt_ps_pool:
                kT_ps = []
                for occ in range(2):
                    kp = kt_ps_pool.tile([128, P], F32, name=f"kT_ps{occ}")
                    kT_ps.append(kp)
                for ph in range(PH):
                    for py in range(PG):
                        r = (ph * PG + py) % 2
                        eng = nc.sync if r == 0 else nc.scalar
                        eng.dma_start(
                            out=v4[10 * py : 10 * py + 10, :, ph, :],
                            in_=x_r[py, :, :, ph, :],
                        )
                    # stage-1 pooling: sum over pw for this ph row
                    nc.vector.reduce_sum(
                        cs3[:100, :, ph], v4[:100, :, ph, :], axis=AX.X
                    )
                    pt_ps = pt_ps_pool.tile([128, P], F32, name="pt_ps", tag="ptp")
                    nc.tensor.transpose(
                        pt_ps[:], cs3[:100, :, ph], ident_f[:100, :100]
                    )
                    pt_sb = pt_sb_pool.tile([128, P], F32, name="pt_sb", tag="pts")
                    nc.vector.tensor_copy(pt_sb[:], pt_ps[:])
                    for occ in range(2):
                        nc.tensor.matmul(
                            kT_ps[occ][:],
                            keyw[:, 128 * occ : 128 * occ + 128],
                            pt_sb[:],
                            start=(ph == 0),
                            stop=(ph == PH - 1),
                        )
                for occ in range(2):
                    nc.vector.tensor_copy(kT_v[:, occ, :], kT_ps[occ][:])

                # ---------------- scores + unnormalized softmax ---------------
                # scores are O(1) here so exp needs no max subtraction; the
                # 1/sum normalization is folded into the y strip copies.
                with tc.tile_pool(name="sc_ps", bufs=2, space="PSUM") as sc_ps_pool:
                    for h in range(HEADS):
                        occ, off = h // 4, (h % 4) * 32
                        sc_ps = sc_ps_pool.tile([128, P], F32, name="sc_ps", tag="sc")
                        nc.tensor.matmul(
                            sc_ps[:100],
                            qT_v[off : off + 32, occ, :],
                            kT_v[off : off + 32, occ, :],
                            start=True, stop=True,
                            tile_position=(off, 0),
                        )
                        nc.scalar.activation(
                            pexp_v[:100, h, :], sc_ps[:100],
                            AF.Exp, bias=zero_t[:100], scale=SCL,
                            accum_out=se8[:100, h : h + 1],
                        )
                    nc.vector.reciprocal(se8[:100], se8[:100])

            # ---------------- y = wts @ v in per-ph strips, store -------------
            with tc.tile_pool(name="wt2_sb", bufs=8) as wt2_sb_pool, \
                 tc.tile_pool(name="y_ps", bufs=6, space="PSUM") as y_ps_pool, \
                 tc.tile_pool(name="strip", bufs=4) as strip_pool:
                wtsT = []
                with tc.tile_pool(name="wt2_ps", bufs=2, space="PSUM") as wt2_ps_pool:
                    for h in range(HEADS):
                        wt2_ps = wt2_ps_pool.tile(
                            [128, P], F32, name="wt2_ps", tag="wt2p"
                        )
                        nc.tensor.transpose(
                            wt2_ps[:100], pexp_v[:100, h, :], ident_f[:100, :100]
                        )
                        wt2_sb = wt2_sb_pool.tile(
                            [128, P], F32, name=f"wt2_sb{h}", tag=f"wt2s{h}"
                        )
                        nc.vector.tensor_copy(wt2_sb[:100], wt2_ps[:100])
                        wtsT.append(wt2_sb)
                for ph in range(PH):
                    strip = strip_pool.tile(
                        [128, C * PH], F32, name="strip", tag="strip"
                    )
                    s3 = strip[:].rearrange("p (c pw) -> p c pw", c=C, pw=PH)
                    for h in range(HEADS):
                        y_ps = y_ps_pool.tile([128, CH * PH], F32, name="y_ps",
                                              tag="y_ps")
                        nc.tensor.matmul(
                            y_ps[:100],
                            wtsT[h][:100],
                            v4[:100, CH * h : CH * (h + 1), ph, :],
                            start=True, stop=True,
                        )
                        # copy + softmax normalization (per-q scale)
                        nc.scalar.mul(
                            s3[:100, CH * h : CH * (h + 1), :], y_ps[:100],
                            se8[:100, h : h + 1],
                        )
                    for py in range(PG):
                        eng = nc.sync if py % 2 == 0 else nc.scalar
                        eng.dma_start(
                            out=out_r[py, :, :, ph, :],
                            in_=s3[10 * py : 10 * py + 10],
                        )

    nc.compile()
    return nc


_NC = None


def _get_nc():
    global _NC
    if _NC is None:
        _NC = build_nc()
    return _NC


def _run(inputs: dict, trace: bool = False):
    from concourse.bass_utils import run_bass_kernel_spmd

    nc = _get_nc()
    per_core = []
    for i in range(NCORES):
        m = {
            "x": np.ascontiguousarray(inputs["x"][i], dtype=np.float32),
            "x_high": np.ascontiguousarray(inputs["x_high"][i], dtype=np.float32),
            "key_w": np.ascontiguousarray(inputs["key_w"], dtype=np.float32),
            "conv_w": np.ascontiguousarray(inputs["conv_w"], dtype=np.float32),
            "bn_gamma": np.ascontiguousarray(inputs["bn_gamma"], dtype=np.float32),
            "bn_beta": np.ascontiguousarray(inputs["bn_beta"], dtype=np.float32),
            "bn_mean": np.ascontiguousarray(inputs["bn_mean"], dtype=np.float32),
            "bn_var": np.ascontiguousarray(inputs["bn_var"], dtype=np.float32),
        }
        per_core.append(m)
    res = run_bass_kernel_spmd(nc, per_core, list(range(NCORES)), trace=trace)
    out = np.stack([res.results[i]["out"] for i in range(NCORES)], axis=0)
    return out, res


def kernel(**inputs) -> np.ndarray:
    out, _ = _run(inputs, trace=False)
    return out
